# revision 27
# baseline (speedup 1.0000x reference)
"""7x7 box blur (reflect padding, depthwise over channels) on TRN2, 8 cores.

Math: out = (1/49) * Bv^T @ X @ Bh per (batch, channel) image, where
Bv == Bh == B is the 512x512 banded 0/1/2 integer matrix encoding the
7-tap box window with reflect boundary folded in.  B is exact in fp16.

Two TensorE passes per image, no explicit transposes:
  pass 1: T1[w, h'] = sum_h X[h, w] * B[h, h']   (vertical blur, output
          transposed -- X block is the stationary lhsT operand)
  pass 2: O[h', w'] = sum_w T1[w, h'] * B[w, w'] (horizontal blur, output
          back in natural layout)

Memory precision: the kernel is HBM-bandwidth-bound (~285 GB/s/core
aggregate), so both HBM sides are fp16: the host pre-casts x to fp16
(the device math was already fp16 internally) and the device stores the
output as fp16, upcast to fp32 on the host.  This halves DMA traffic
per core from 25.2 MB to 12.6 MB.  Total rel err stays ~5e-4.

Sharding: pure data parallel, batch dim split 32 -> 8 cores x 4.
Each core processes 12 images (4 batches x 3 channels) of 512x512.
"""

import numpy as np
from contextlib import ExitStack

H = W = 512
IMGS = 12          # images per core: 4 batches * 3 channels
N_CORES = 8
# output column windows (h' for pass 1, w' for pass 2)
WINS = [(0, 122), (122, 244), (244, 366), (366, 488), (488, 512)]
# input row-window of each pass-2 lhsT block (w range covering taps of WINS[j])
WBLK = [(0, 125), (119, 247), (241, 369), (363, 491), (485, 512)]
# pass-1 output windows (h'): three ~170-wide windows; each gets
# contributions from exactly two 128-row input blocks
P1_WINS = [(0, 170), (170, 340), (340, 512)]
# pass-1 matmul list: (input 128-row block b, output window index) in an order
# that keeps each PSUM write region homogeneous (write-then-accumulate)
P1_MMS = [(0, 0), (1, 0), (1, 1), (2, 1), (2, 2), (3, 2)]
P1_STRIDE = 172  # column stride of packed pass-1 rhs chunks

# --- r2-interleaved layout (ilv=2): partition p holds row pairs (2p, 2p+1),
# so every HBM<->SBUF descriptor covers 2 consecutive rows = 2 KB fp16,
# halving per-queue DMA descriptor-processing time.  Input h-blocks become
# (half s, parity r): rows 256*s + 2p + r.  Pass-1 h' regions: [0,259) from
# the s=0 blocks, [259,512) from s=1, plus a 6-wide boundary strip [253,259)
# accumulating the s=1 contribution on top of the s=0 result.
# entries: (s, r, a, b, start, stop).  The [259,512) group runs FIRST:
# start=True marks the whole 2KB PSUM zero-region pending-zero, so the
# second start must precede nothing it would invalidate -- after it, the
# [0,259) group's writes clear pending state over bytes the boundary
# strip then accumulates into.
R2_P1_MMS = [
    (1, 0, 259, 512, True, False),
    (1, 1, 259, 512, False, True),
    (0, 0, 0, 259, True, False),
    (0, 1, 0, 259, False, True),
    (1, 0, 253, 259, False, False),
    (1, 1, 253, 259, False, True),
]
R2_CHUNKS = [(0, 0), (0, 1), (1, 0), (1, 1)]  # pass-2 output (s, r) chunks

_STATE: dict = {}


def _band_matrix() -> np.ndarray:
    """B[i, j] = multiplicity of input row i among the 7 reflect-padded taps
    of output row j."""
    B = np.zeros((512, 512), np.float32)
    j = np.arange(512)
    for d in range(-3, 4):
        i = np.abs(j + d)
        i = np.where(i > 511, 1022 - i, i)
        np.add.at(B, (i, j), 1.0)
    return B


def _build_consts(ilv: int = 0):
    B = _band_matrix()
    if ilv == 2:
        # pass-1 rhs chunks packed tight in R2_P1_MMS order; block (s, r)
        # partition p maps to input row 256*s + 2p + r.
        width = sum(b - a for (_, _, a, b, _, _) in R2_P1_MMS)
        bv = np.zeros((128, width), np.float16)
        off = 0
        for (s, r, a, b, _, _) in R2_P1_MMS:
            bv[:, off:off + b - a] = B[256 * s + r: 256 * (s + 1): 2, a:b]
            off += b - a
    else:
        # pass-1 rhs: for each (block b, window): rows 128b..128b+128 of B,
        # cols P1_WINS[win], padded to width P1_STRIDE, laid side by side.
        bv = np.zeros((128, P1_STRIDE * len(P1_MMS)), np.float16)
        for k, (b, win) in enumerate(P1_MMS):
            s, e = P1_WINS[win]
            bv[:, P1_STRIDE * k: P1_STRIDE * k + (e - s)] = \
                B[128 * b: 128 * (b + 1), s:e]
    # pass-2 rhs: for window j, rows are remapped to block j's partition
    # space (partition p = global w row WBLK[j][0] + p), cols WINS[j].
    bh = np.zeros((128, 512), np.float16)
    for jw, ((ws, we), (s, e)) in enumerate(zip(WBLK, WINS)):
        bh[: we - ws, s:e] = B[ws:we, s:e]
    return bv, bh


def _build_nc(repeat: int = 1, loop_repeat: int = 0, group: int = -1,
              variant: str = "full", load_eng: str = "gp",
              store_eng: str = "sp", balance: int = 1, deep: int = 0,
              ilv: int = 0):
    """loop_repeat > 0 wraps the whole 12-image pipeline in a runtime
    For_i loop executing it that many times -- used only for timing (one
    NEFF dispatch, loop_repeat x the device work).
    variant: 'full' | 'dma' (loads+stores only) | 'nostore' (loads+compute)
    | 'load' (loads only) | 'store' (stores only) -- diagnostic builds.
    load_eng / store_eng: which queue issues the transfer --
    'gp' (SWDGE), 'sp' (qSP HWDGE), 'act' (qActivation HWDGE)."""
    do_dma = variant in ("full", "dma", "nostore", "load")
    do_compute = variant in ("full", "nostore", "comp")
    do_store = variant in ("full", "dma", "store")
    import concourse.tile as tile
    from concourse import bacc, mybir

    f16 = mybir.dt.float16
    f32 = mybir.dt.float32

    nc = bacc.Bacc("TRN2", target_bir_lowering=False, debug=False,
                   enable_asserts=True)
    bv_cols = (sum(b - a for (_, _, a, b, _, _) in R2_P1_MMS) if ilv == 2
               else P1_STRIDE * len(P1_MMS))
    x_ap = nc.dram_tensor("x", [IMGS, H, W], f16, kind="ExternalInput").ap()
    bv_ap = nc.dram_tensor("bv", [128, bv_cols], f16,
                           kind="ExternalInput").ap()
    bh_ap = nc.dram_tensor("bh", [128, 512], f16, kind="ExternalInput").ap()
    out_ap = nc.dram_tensor("out", [IMGS, H, W], f16, kind="ExternalOutput").ap()

    def io_view(ap, gstart, GROUP):
        # HBM <-> SBUF view of a group of images; ilv=2 pairs 2 consecutive
        # rows per partition for 2 KB descriptors.
        if ilv == 2:
            return ap[gstart:gstart + GROUP].rearrange(
                "i (s p r) w -> p i s r w", s=2, p=128, r=2)
        return ap[gstart:gstart + GROUP].rearrange(
            "i (s p) w -> p i s w", p=128)

    def dma_eng(which):
        return {"gp": nc.gpsimd, "sp": nc.sync, "act": nc.scalar}[which]

    with tile.TileContext(nc) as tc, ExitStack() as ctx:
        cpool = ctx.enter_context(tc.tile_pool(name="const", bufs=1))
        xpool = ctx.enter_context(
            tc.tile_pool(name="xin", bufs=5 if deep else 3))
        t1ppool = ctx.enter_context(tc.tile_pool(name="t1p", bufs=4, space="PSUM"))
        t1pool = ctx.enter_context(tc.tile_pool(name="t1", bufs=10))
        oppool = ctx.enter_context(tc.tile_pool(name="opsum", bufs=4, space="PSUM"))
        outpool = ctx.enter_context(
            tc.tile_pool(name="osb", bufs=4 if deep else 3))

        bv = cpool.tile([128, bv_cols], f16)
        nc.sync.dma_start(bv[:], bv_ap[:])
        bh = cpool.tile([128, 512], f16)
        nc.sync.dma_start(bh[:], bh_ap[:])

        dummy_osb = None
        if variant in ("dma", "store"):
            dummy_osb = cpool.tile([128, group * 4 * W], f16)
            nc.vector.memset(dummy_osb[:], 0.0)
        dummy_x = None
        if variant == "comp":
            # compute-only: all groups read one zeroed const tile
            dummy_x = cpool.tile([128, max(2, abs(group)) * 4 * W], f16)
            nc.vector.memset(dummy_x[:], 0.0)
        loop_ctx = (tc.For_i(0, loop_repeat, 1,
                             hint_engines=(mybir.EngineType.PE,))
                    if loop_repeat > 0 else None)
        if loop_ctx is not None:
            ctx.enter_context(loop_ctx)
        if group == 0:
            sizes = [1, 1, 2, 2, 2, 2, 1, 1]
        elif group == -1:
            sizes = [1, 1, 1, 2, 2, 2, 1, 1, 1]
        else:
            sizes = [group] * (IMGS // group)
        assert sum(sizes) == IMGS
        sched = []
        for rep in range(repeat):
            s0 = 0
            for gnum, sz in enumerate(sizes):
                for gi in range(sz):
                    sched.append((s0, sz, gi, gnum))
                s0 += sz
        ROT = ["gp", "sp", "act"]
        for (gstart, GROUP, g, gnum) in sched:
            img = gstart + g
            if g == 0:
                # load one image group; layout:
                # xtg[:, 2048*i + 512*b + w] = x[img+i, 128*b + p, w]
                if variant == "comp":
                    xtg = dummy_x[:, :GROUP * 4 * W]
                else:
                    xtg = xpool.tile([128, max(2, abs(group)) * 4 * W], f16,
                                     tag="xt")
                    xtg = xtg[:, :GROUP * 4 * W]
                if do_dma:
                    src = io_view(x_ap, gstart, GROUP)
                    if load_eng == "rot":
                        dma_eng(ROT[gnum % 3]).dma_start(xtg[:], src)
                    elif len(load_eng) > 3:  # e.g. 'gpsp', 'gpact', 'spact'
                        e0, e1 = ((load_eng[:2], load_eng[2:])
                                  if load_eng[:2] in ("gp", "sp")
                                  else (load_eng[:3], load_eng[3:]))
                        smid = 1 if ilv == 2 else 2
                        for i in range(GROUP):
                            dma_eng(e0).dma_start(
                                xtg[:, i * 4 * W: i * 4 * W + 2 * W],
                                src[:, i, :smid])
                            dma_eng(e1).dma_start(
                                xtg[:, i * 4 * W + 2 * W: (i + 1) * 4 * W],
                                src[:, i, smid:])
                    else:
                        dma_eng(load_eng).dma_start(xtg[:], src)
            xt = xtg[:, g * 4 * W:(g + 1) * 4 * W]
            if not do_compute:
                if g == GROUP - 1 and do_store:
                    seng = (ROT[(gnum + 1) % 3] if store_eng == "rot"
                            else store_eng)
                    dma_eng(seng).dma_start(
                        io_view(out_ap, gstart, GROUP),
                        dummy_osb[:, :GROUP * 4 * W])
                continue

            # pass 1: T1[w, h'] per overlapping w-block j
            t1_tiles = []
            for jw, (ws, we) in enumerate(WBLK):
                mj = we - ws
                t1p = t1ppool.tile([128, 512], f32, tag="t1p")
                if ilv == 2:
                    off = 0
                    for (s, r, a, b, st, sp) in R2_P1_MMS:
                        nc.tensor.matmul(
                            t1p[:mj, a:b],
                            lhsT=xt[:, (s * 2 + r) * W + ws:
                                    (s * 2 + r) * W + we],
                            rhs=bv[:, off:off + b - a],
                            start=st, stop=sp,
                            # the boundary-strip mms re-open [253,259) to
                            # accumulate the s=1 rows onto the closed s=0
                            # group; plain PSUM read-modify-write on HW
                            skip_group_check=(a, b) == (253, 259) and not st,
                        )
                        off += b - a
                else:
                    for k, (b, win) in enumerate(P1_MMS):
                        s, e = P1_WINS[win]
                        nc.tensor.matmul(
                            t1p[:mj, s:e],
                            lhsT=xt[:, b * W + ws: b * W + we],
                            rhs=bv[:, P1_STRIDE * k: P1_STRIDE * k + (e - s)],
                            start=(k == 0), stop=(k == len(P1_MMS) - 1),
                        )
                t1 = t1pool.tile([128, 512], f16, tag="t1")
                # PSUM->SBUF fp16 cast, alternating ACT/DVE to balance load
                if balance and (jw + img) % 2 == 0:
                    nc.vector.tensor_scalar_mul(t1[:mj, :], t1p[:mj, :], 1.0)
                else:
                    nc.scalar.copy(t1[:mj, :], t1p[:mj, :])
                t1_tiles.append((t1, mj))

            # pass 2: O[h', w'] per 128-row h' chunk c
            if g == 0:
                osbg = outpool.tile([128, max(2, abs(group)) * 4 * W], f16,
                                    tag="osb")
                osbg = osbg[:, :GROUP * 4 * W]
            osb = osbg[:, g * 4 * W:(g + 1) * 4 * W]
            for c in range(4):
                op = oppool.tile([128, 512], f32, tag="op")
                for jw, (t1, mj) in enumerate(t1_tiles):
                    s, e = WINS[jw]
                    if ilv == 2:
                        # chunk c=(sc, rc): output rows 256*sc + 2p + rc,
                        # i.e. t1 cols strided 2 starting at 256*sc + rc
                        sc, rc = R2_CHUNKS[c]
                        lhsT = t1[:mj].rearrange(
                            "q (s p r) -> q s r p", s=2, p=128, r=2)[:, sc, rc, :]
                    else:
                        lhsT = t1[:mj, c * 128: (c + 1) * 128]
                    nc.tensor.matmul(
                        op[:, s:e],
                        lhsT=lhsT,
                        rhs=bh[:mj, s:e],
                        start=(jw == 0), stop=(jw == len(t1_tiles) - 1),
                    )
                # final 1/49 scale + PSUM->SBUF fp16, split across DVE and ACT
                dst = osb[:, c * W: (c + 1) * W]
                if c % 2 == 0:
                    nc.vector.tensor_scalar_mul(dst, op[:], 1.0 / 49.0)
                else:
                    nc.scalar.mul(dst, op[:], 1.0 / 49.0)
            if g == GROUP - 1 and do_store:
                out_view = io_view(out_ap, gstart, GROUP)
                if store_eng == "rot":
                    dma_eng(ROT[(gnum + 1) % 3]).dma_start(
                        out_view, osbg[:, :GROUP * 4 * W])
                elif len(store_eng) > 3:
                    e0, e1 = ((store_eng[:2], store_eng[2:])
                              if store_eng[:2] in ("gp", "sp")
                              else (store_eng[:3], store_eng[3:]))
                    smid = 1 if ilv == 2 else 2
                    for i in range(GROUP):
                        dma_eng(e0).dma_start(
                            out_view[:, i, :smid],
                            osbg[:, i * 4 * W: i * 4 * W + 2 * W])
                        dma_eng(e1).dma_start(
                            out_view[:, i, smid:],
                            osbg[:, i * 4 * W + 2 * W: (i + 1) * 4 * W])
                else:
                    dma_eng(store_eng).dma_start(
                        out_view, osbg[:, :GROUP * 4 * W])

    nc.compile()
    return nc


def _get_state(repeat: int = 1, loop_repeat: int = 0, group: int = -1,
               variant: str = "full", load_eng: str = "gp",
               store_eng: str = "sp", balance: int = 1, deep: int = 0,
               ilv: int = 2):
    key = ("nc", repeat, loop_repeat, group, variant, load_eng, store_eng,
           balance, deep, ilv)
    if key not in _STATE:
        _STATE[key] = _build_nc(repeat, loop_repeat, group, variant,
                                load_eng, store_eng, balance, deep, ilv)
    ckey = ("consts", ilv)
    if ckey not in _STATE:
        _STATE[ckey] = _build_consts(ilv)
    bv, bh = _STATE[ckey]
    return {"nc": _STATE[key], "bv": bv, "bh": bh}


def _make_runner(repeat: int = 1, loop_repeat: int = 0, group: int = -1,
                 variant: str = "full", load_eng: str = "gp",
                 store_eng: str = "sp", balance: int = 1, deep: int = 0,
                 ilv: int = 2):
    """Cached 8-core sharded jit over the bass program (mirrors
    bass2jax.run_bass_via_pjrt's multicore path, minus buffer donation so
    the compiled fn can be invoked repeatedly for timing)."""
    rkey = ("runner", repeat, loop_repeat, group, variant, load_eng,
            store_eng, balance, deep, ilv)
    if rkey in _STATE:
        return _STATE[rkey]
    import jax
    import jax.numpy as jnp
    from jax.sharding import Mesh, PartitionSpec
    from jax.experimental.shard_map import shard_map
    from concourse import bass2jax, mybir

    st = _get_state(repeat, loop_repeat, group, variant, load_eng,
                    store_eng, balance, deep, ilv)
    nc = st["nc"]
    bass2jax.install_neuronx_cc_hook()

    partition_name = (nc.partition_id_tensor.name
                      if nc.partition_id_tensor else None)
    in_names, out_names, out_avals = [], [], []
    for alloc in nc.m.functions[0].allocations:
        if not isinstance(alloc, mybir.MemoryLocationSet):
            continue
        name = alloc.memorylocations[0].name
        if alloc.kind == "ExternalInput":
            if name != partition_name:
                in_names.append(name)
        elif alloc.kind == "ExternalOutput":
            out_names.append(name)
            out_avals.append(jax.core.ShapedArray(
                tuple(alloc.tensor_shape), mybir.dt.np(alloc.dtype)))
    n_params = len(in_names)
    all_names = in_names + out_names
    if partition_name is not None:
        all_names = all_names + [partition_name]

    def _body(*args):
        operands = list(args)
        if partition_name is not None:
            operands.append(bass2jax.partition_id_tensor())
        outs = bass2jax._bass_exec_p.bind(
            *operands,
            out_avals=tuple(out_avals),
            in_names=tuple(all_names),
            out_names=tuple(out_names),
            lowering_input_output_aliases=(),
            sim_require_finite=True,
            sim_require_nnan=True,
            nc=nc,
        )
        return tuple(outs)

    devices = jax.devices()[:N_CORES]
    mesh = Mesh(np.asarray(devices), ("core",))
    n_outs = len(out_names)
    sharded = jax.jit(shard_map(
        _body, mesh=mesh,
        in_specs=(PartitionSpec("core"),) * (n_params + n_outs),
        out_specs=(PartitionSpec("core"),) * n_outs,
        check_rep=False))
    _STATE[rkey] = (sharded, in_names, out_names, out_avals)
    return _STATE[rkey]


def _concat_inputs(x: np.ndarray):
    st = _get_state()
    _, in_names, out_names, out_avals = _make_runner()
    B, C = x.shape[0], x.shape[1]
    per = B // N_CORES
    shards = {
        "x": np.ascontiguousarray(
            x.reshape(N_CORES, per * C, H, W)).astype(np.float16),
        "bv": np.broadcast_to(st["bv"], (N_CORES,) + st["bv"].shape),
        "bh": np.broadcast_to(st["bh"], (N_CORES,) + st["bh"].shape),
    }
    concat_in = [
        np.ascontiguousarray(shards[n]).reshape(
            (N_CORES * shards[n].shape[1],) + shards[n].shape[2:])
        for n in in_names]
    concat_zeros = [
        np.zeros((N_CORES * a.shape[0],) + a.shape[1:], a.dtype)
        for a in out_avals]
    return concat_in, concat_zeros


def kernel(x: np.ndarray) -> np.ndarray:
    from concourse import bass_utils
    st = _get_state()
    x = np.asarray(x, np.float32)
    B, C = x.shape[0], x.shape[1]
    per = B // N_CORES
    in_maps = []
    for i in range(N_CORES):
        shard = np.ascontiguousarray(
            x[i * per:(i + 1) * per].reshape(per * C, H, W)).astype(
                np.float16)
        in_maps.append({"x": shard, "bv": st["bv"], "bh": st["bh"]})
    res = bass_utils.run_bass_kernel_spmd(
        st["nc"], in_maps, core_ids=list(range(N_CORES)))
    out = np.concatenate(
        [res.results[i]["out"].reshape(per, C, H, W)
         for i in range(N_CORES)], axis=0)
    return np.ascontiguousarray(out).astype(np.float32)


def benchmark(x: np.ndarray, iters: int = 30) -> float:
    """Returns steady-state per-invocation wall time in ns for the 8-core
    SPMD execution (inputs sharded and resident on their devices; outputs
    chained into the next call's scratch operand so iterations pipeline
    without host round-trips)."""
    import time
    import jax
    from jax.sharding import Mesh, NamedSharding, PartitionSpec

    x = np.asarray(x, np.float32)
    sharded, in_names, out_names, out_avals = _make_runner()
    concat_in, concat_zeros = _concat_inputs(x)
    devices = jax.devices()[:N_CORES]
    mesh = Mesh(np.asarray(devices), ("core",))
    shard0 = NamedSharding(mesh, PartitionSpec("core"))
    dev_in = [jax.device_put(a, shard0) for a in concat_in]
    dev_zero = [jax.device_put(a, shard0) for a in concat_zeros]
    # warm up (compiles on first call)
    outs = sharded(*dev_in, *dev_zero)
    jax.block_until_ready(outs)
    # chained steady-state loop: prior outputs feed the scratch-out slots
    t0 = time.perf_counter()
    for _ in range(iters):
        outs = sharded(*dev_in, *outs)
    jax.block_until_ready(outs)
    dt = (time.perf_counter() - t0) / iters
    return dt * 1e9
